# revision 15
# baseline (speedup 1.0000x reference)
"""Trainium2 Bass kernel for nn_EnhancedRecurrentGCN (K=1 DConv DCRNN stack).

Math (h0 == 0 collapses each DCRNN cell; the r-gate is multiplied by zero):
    a  = x @ W1z;  s  = sigmoid(-a)         (= 1 - sigmoid(a))
    b  = x @ W1h;  u  = tanh(b)
    h1 = relu(s * u) = s * max(u, 0)        (s > 0)
    a2 = h1 @ W2z; s2 = sigmoid(-a2)
    b2 = h1 @ W2h; u2 = tanh(b2)
    h2 = s2 * max(u2, 0)
    h3 = relu(h2 @ W3 + b3)
    y  = h3 @ W4 + b4
with W1z = (w_z1[0,0]+w_z1[1,0])[:256] etc.  edge_index/edge_weight unused.

Sharding: pure data parallelism over nodes; x padded 100000 -> 102400 rows,
12800 nodes/core, fed pre-transposed (xt = x_shard.T) so the contraction dim
is the SBUF partition dim.  All device tensors are feature-major.

Packing tricks (all fp32r, full precision):
 - Layer 2 outputs of two adjacent 512-node subtiles are packed onto psum
   partitions 0:64 / 64:128 via zero-embedded M=128 weights accumulating into
   one bank -> full-lane sigmoid/tanh/mul, one call per 1024 nodes.
 - Layer 3 outputs of up to 8 subtiles land on partitions 16j:16j+16 of one
   psum bank via column-embedded W3 copies accumulating -> relu once/group.
 - Layer 4 for a whole group is ONE matmul with a block-diagonal W4 embed;
   its [group,512] output is evicted once.
"""

import sys

if "/opt/trn_rl_repo" not in sys.path:
    sys.path.insert(0, "/opt/trn_rl_repo")

from contextlib import ExitStack

import numpy as np

import concourse.mybir as mybir
import concourse.tile as tile
from concourse import bacc
from concourse.bass_utils import run_bass_kernel_spmd

N_CORES = 8
PAD_NODES = 102400
SHARD = PAD_NODES // N_CORES  # 12800
TN = 512                      # nodes per subtile (psum bank width, fp32)
CHUNK = 2560                  # nodes per input DMA chunk (1.28 MB per half)
GROUP = 8                     # subtiles per L3/L4 pack group (16*8 = 128)

F32 = mybir.dt.float32
F32R = mybir.dt.float32r
AF = mybir.ActivationFunctionType
OP = mybir.AluOpType


def build_nc(shard=SHARD):
    assert shard % TN == 0
    n_sub = shard // TN
    groups = [list(range(g, min(g + GROUP, n_sub)))
              for g in range(0, n_sub, GROUP)]
    n_grp = len(groups)
    chunks = []
    c = 0
    while c < shard:
        w = min(CHUNK, shard - c)
        chunks.append((c, w))
        c += w

    nc = bacc.Bacc(None)

    xt = nc.declare_dram_parameter("xt", [256, shard], F32R, isOutput=False)
    w1z = nc.declare_dram_parameter("w1z", [256, 128], F32R, isOutput=False)
    w1h = nc.declare_dram_parameter("w1h", [256, 128], F32R, isOutput=False)
    w2ze = nc.declare_dram_parameter("w2ze", [2, 128, 128], F32R, isOutput=False)
    w2he = nc.declare_dram_parameter("w2he", [2, 128, 128], F32R, isOutput=False)
    w3e = nc.declare_dram_parameter("w3e", [GROUP, 128, 128], F32R, isOutput=False)
    w4e = nc.declare_dram_parameter("w4e", [128, GROUP], F32R, isOutput=False)
    nbz1 = nc.declare_dram_parameter("nbz1", [128, 1], F32, isOutput=False)
    bh1 = nc.declare_dram_parameter("bh1", [128, 1], F32, isOutput=False)
    nbz2p = nc.declare_dram_parameter("nbz2p", [128, 1], F32, isOutput=False)
    bh2p = nc.declare_dram_parameter("bh2p", [128, 1], F32, isOutput=False)
    b3sp = nc.declare_dram_parameter("b3sp", [128, 1], F32, isOutput=False)
    b4bc = nc.declare_dram_parameter("b4bc", [GROUP, 1], F32, isOutput=False)
    out = nc.declare_dram_parameter("out", [GROUP, TN * n_grp], F32, isOutput=True)

    with ExitStack() as ctx:
        tc = ctx.enter_context(tile.TileContext(nc))
        wp = ctx.enter_context(tc.tile_pool(name="weights", bufs=1))
        xp = ctx.enter_context(tc.tile_pool(name="x", bufs=2))
        ap = ctx.enter_context(tc.tile_pool(name="acts", bufs=2))
        ob = ctx.enter_context(tc.tile_pool(name="outbuf", bufs=1))
        # PSUM: exactly 8 banks
        pz1 = ctx.enter_context(tc.tile_pool(name="pz1", bufs=1, space="PSUM"))
        pt1 = ctx.enter_context(tc.tile_pool(name="pt1", bufs=1, space="PSUM"))
        pz2 = ctx.enter_context(tc.tile_pool(name="pz2", bufs=1, space="PSUM"))
        pt2 = ctx.enter_context(tc.tile_pool(name="pt2", bufs=1, space="PSUM"))
        ph3 = ctx.enter_context(tc.tile_pool(name="ph3", bufs=1, space="PSUM"))
        po = ctx.enter_context(tc.tile_pool(name="po", bufs=1, space="PSUM"))

        # --- stage weights + biases into SBUF (once) ---
        w1z_sb = [wp.tile([128, 128], F32R, name=f"w1z_sb{k}") for k in range(2)]
        w1h_sb = [wp.tile([128, 128], F32R, name=f"w1h_sb{k}") for k in range(2)]
        for k in range(2):
            nc.sync.dma_start(w1z_sb[k][:], w1z[128 * k:128 * (k + 1), :])
            nc.sync.dma_start(w1h_sb[k][:], w1h[128 * k:128 * (k + 1), :])
        w2ze_sb = [wp.tile([128, 128], F32R, name=f"w2ze_sb{v}") for v in range(2)]
        w2he_sb = [wp.tile([128, 128], F32R, name=f"w2he_sb{v}") for v in range(2)]
        for v in range(2):
            nc.sync.dma_start(w2ze_sb[v][:], w2ze[v])
            nc.sync.dma_start(w2he_sb[v][:], w2he[v])
        w3e_sb = [wp.tile([128, 128], F32R, name=f"w3e_sb{j}") for j in range(GROUP)]
        for j in range(GROUP):
            nc.sync.dma_start(w3e_sb[j][:], w3e[j])
        w4e_sb = wp.tile([128, GROUP], F32R, name="w4e_sb")
        nc.sync.dma_start(w4e_sb[:], w4e[:])
        bias_sb = {}
        for nm, dram, p in [("nbz1", nbz1, 128), ("bh1", bh1, 128),
                            ("nbz2p", nbz2p, 128), ("bh2p", bh2p, 128),
                            ("b3sp", b3sp, 128), ("b4bc", b4bc, GROUP)]:
            t = wp.tile([p, 1], F32, name=f"{nm}_sb")
            nc.sync.dma_start(t[:], dram[:])
            bias_sb[nm] = t

        out_sb = ob.tile([GROUP, TN * n_grp], F32)
        nc.vector.memset(out_sb[:], 0.0)

        x_tiles = {}

        def ensure_chunk(ci):
            if ci in x_tiles or ci >= len(chunks):
                return
            c0, cw = chunks[ci]
            xa = xp.tile([128, cw], F32R, tag="xa", name=f"xa{ci}")
            xb = xp.tile([128, cw], F32R, tag="xb", name=f"xb{ci}")
            nc.sync.dma_start(xa[:], xt[0:128, c0:c0 + cw])
            nc.sync.dma_start(xb[:], xt[128:256, c0:c0 + cw])
            x_tiles[ci] = (xa, xb)

        def x_slice(s):
            """(xa_tile, xb_tile, col slice) for subtile s."""
            ci = (s * TN) // CHUNK
            ensure_chunk(ci)
            ensure_chunk(ci + 1)  # prefetch
            off = s * TN - chunks[ci][0]
            return x_tiles[ci][0], x_tiles[ci][1], slice(off, off + TN)

        h3p_cur = [None]

        # macros: pairs of subtiles (+ tail single)
        macros = [(2 * m, 2) for m in range(n_sub // 2)]
        if n_sub % 2:
            macros.append((n_sub - 1, 1))

        for s0, nsub in macros:
            mw = nsub * TN
            # ---- layer 1 ----
            zpre = pz1.tile([128, mw], F32, tag="zpre", name=f"zpre{s0}")
            tpre = pt1.tile([128, mw], F32, tag="tpre", name=f"tpre{s0}")
            for i in range(nsub):
                xa, xb, sl = x_slice(s0 + i)
                d = slice(i * TN, (i + 1) * TN)
                nc.tensor.matmul(zpre[:, d], w1z_sb[0][:], xa[:, sl],
                                 start=True, stop=False, skip_group_check=True)
                nc.tensor.matmul(zpre[:, d], w1z_sb[1][:], xb[:, sl],
                                 start=False, stop=True, skip_group_check=True)
                nc.tensor.matmul(tpre[:, d], w1h_sb[0][:], xa[:, sl],
                                 start=True, stop=False, skip_group_check=True)
                nc.tensor.matmul(tpre[:, d], w1h_sb[1][:], xb[:, sl],
                                 start=False, stop=True, skip_group_check=True)
            s1t = ap.tile([128, mw], F32, tag="s1t", name=f"s1t{s0}")
            nc.scalar.activation(s1t[:], zpre[:], AF.Sigmoid,
                                 bias=bias_sb["nbz1"][:], scale=-1.0)
            u1t = ap.tile([128, mw], F32, tag="u1t", name=f"u1t{s0}")
            nc.scalar.activation(u1t[:], tpre[:], AF.Tanh,
                                 bias=bias_sb["bh1"][:], scale=1.0)
            h1t = ap.tile([128, mw], F32R, tag="h1t", name=f"h1t{s0}")
            nc.vector.scalar_tensor_tensor(h1t[:], u1t[:], 0.0, s1t[:],
                                           op0=OP.max, op1=OP.mult)

            # ---- layer 2 (partition-packed pair via zero-embedded weights) ----
            z2p = pz2.tile([128, TN], F32, tag="z2p", name=f"z2p{s0}")
            t2p = pt2.tile([128, TN], F32, tag="t2p", name=f"t2p{s0}")
            for i in range(nsub):
                nc.tensor.matmul(z2p[:], w2ze_sb[i][:],
                                 h1t[:, i * TN:(i + 1) * TN],
                                 start=(i == 0), stop=(i == nsub - 1),
                                 skip_group_check=True)
            for i in range(nsub):
                nc.tensor.matmul(t2p[:], w2he_sb[i][:],
                                 h1t[:, i * TN:(i + 1) * TN],
                                 start=(i == 0), stop=(i == nsub - 1),
                                 skip_group_check=True)
            s2t = ap.tile([128, TN], F32, tag="s2t", name=f"s2t{s0}")
            nc.scalar.activation(s2t[:], z2p[:], AF.Sigmoid,
                                 bias=bias_sb["nbz2p"][:], scale=-1.0)
            u2t = ap.tile([128, TN], F32, tag="u2t", name=f"u2t{s0}")
            nc.scalar.activation(u2t[:], t2p[:], AF.Tanh,
                                 bias=bias_sb["bh2p"][:], scale=1.0)
            h2t = ap.tile([128, TN], F32R, tag="h2t", name=f"h2t{s0}")
            nc.vector.scalar_tensor_tensor(h2t[:], u2t[:], 0.0, s2t[:],
                                           op0=OP.max, op1=OP.mult)
            h2ts = [(h2t, i) for i in range(nsub)]

            # ---- layer 3: accumulate into the group's packed psum bank ----
            for i in range(nsub):
                s = s0 + i
                h2t, half = h2ts[i]
                g = s // GROUP
                j = s - groups[g][0]
                if j == 0:
                    h3p_cur[0] = ph3.tile([128, TN], F32, tag="h3p",
                                          name=f"h3p{g}")
                last = (j == len(groups[g]) - 1)
                # w3e_sb[j] is zero outside rows 64*(j%2):+64 / cols 16j:+16,
                # so a full-K base-0 matmul picks out this subtile's half of
                # the packed h2 tile.  (Row-offset operands crash the HW.)
                nc.tensor.matmul(
                    h3p_cur[0][:], w3e_sb[j][:], h2t[:],
                    start=(j == 0), stop=last, skip_group_check=True)
                if last:
                    gs = len(groups[g])
                    h3s = ap.tile([128, TN], F32R, tag="h3s", name=f"h3s{g}")
                    # h3 = max(h3pre + b3, 0)
                    nc.vector.tensor_scalar(
                        h3s[0:16 * gs, :], h3p_cur[0][0:16 * gs, :],
                        bias_sb["b3sp"][0:16 * gs, :], 0.0,
                        op0=OP.add, op1=OP.max)
                    # ---- layer 4: one matmul for the whole group ----
                    opre = po.tile([GROUP, TN], F32, tag="opre", name=f"op{g}")
                    nc.tensor.matmul(opre[0:gs, :], w4e_sb[0:16 * gs, 0:gs],
                                     h3s[0:16 * gs, :], start=True, stop=True,
                                     skip_group_check=True)
                    nc.vector.tensor_scalar(
                        out_sb[0:gs, g * TN:(g + 1) * TN], opre[0:gs, :],
                        bias_sb["b4bc"][0:gs, :], None, op0=OP.add)

        nc.sync.dma_start(out[:], out_sb[:])

    nc.compile()
    return nc


_NC_CACHE = {}


def _get_nc(shard=SHARD):
    if shard not in _NC_CACHE:
        _NC_CACHE[shard] = build_nc(shard)
    return _NC_CACHE[shard]


def make_in_maps(x, w_z1, b_z1, w_r1, b_r1, w_h1, b_h1,
                 w_z2, b_z2, w_r2, b_r2, w_h2, b_h2,
                 w_lin1, b_lin1, w_lin2, b_lin2,
                 n_cores=N_CORES, shard=SHARD):
    f = np.float32
    w1z = np.ascontiguousarray((np.asarray(w_z1)[0, 0] + np.asarray(w_z1)[1, 0])[:256], f)
    w1h = np.ascontiguousarray((np.asarray(w_h1)[0, 0] + np.asarray(w_h1)[1, 0])[:256], f)
    w2z = np.asarray((np.asarray(w_z2)[0, 0] + np.asarray(w_z2)[1, 0])[:128], f)
    w2h = np.asarray((np.asarray(w_h2)[0, 0] + np.asarray(w_h2)[1, 0])[:128], f)
    w3 = np.asarray(w_lin1, f)
    w4 = np.asarray(w_lin2, f)

    w2ze = np.zeros((2, 128, 128), f)
    w2he = np.zeros((2, 128, 128), f)
    for v in range(2):
        w2ze[v][:, 64 * v:64 * v + 64] = w2z
        w2he[v][:, 64 * v:64 * v + 64] = w2h
    w3e = np.zeros((GROUP, 128, 128), f)
    for j in range(GROUP):
        h = 64 * (j % 2)
        w3e[j][h:h + 64, 16 * j:16 * j + 16] = w3
    w4e = np.zeros((128, GROUP), f)
    for j in range(GROUP):
        w4e[16 * j:16 * j + 16, j] = w4[:, 0]

    bz2 = np.asarray(b_z2, f)
    bh2 = np.asarray(b_h2, f)
    b3 = np.asarray(b_lin1, f)
    common = {
        "w1z": w1z, "w1h": w1h, "w2ze": w2ze, "w2he": w2he,
        "w3e": w3e, "w4e": w4e,
        "nbz1": np.ascontiguousarray(-np.asarray(b_z1, f).reshape(128, 1)),
        "bh1": np.ascontiguousarray(np.asarray(b_h1, f).reshape(128, 1)),
        "nbz2p": np.ascontiguousarray(-np.tile(bz2, 2).reshape(128, 1)),
        "bh2p": np.ascontiguousarray(np.tile(bh2, 2).reshape(128, 1)),
        "b3sp": np.ascontiguousarray(np.tile(b3, GROUP).reshape(128, 1)),
        "b4bc": np.full((GROUP, 1), np.asarray(b_lin2, f).reshape(-1)[0], f),
    }
    x = np.asarray(x, f)
    n = x.shape[0]
    pad = n_cores * shard
    xpad = np.zeros((pad, 256), f)
    xpad[:n] = x
    shards = xpad.reshape(n_cores, shard, 256)
    return [dict(common, xt=np.ascontiguousarray(shards[i].T))
            for i in range(n_cores)]


def unscramble(res, n_cores=N_CORES, shard=SHARD):
    """res: per-core out arrays [GROUP, TN*n_grp] -> flat [n_cores*shard]."""
    n_sub = shard // TN
    full = np.empty(n_cores * shard, np.float32)
    for i in range(n_cores):
        o = res[i]
        for g in range((n_sub + GROUP - 1) // GROUP):
            gs = min(GROUP, n_sub - g * GROUP)
            for j in range(gs):
                s = g * GROUP + j
                full[i * shard + s * TN:i * shard + (s + 1) * TN] = \
                    o[j, g * TN:(g + 1) * TN]
    return full


def kernel(x, edge_index=None, edge_weight=None,
           w_z1=None, b_z1=None, w_r1=None, b_r1=None, w_h1=None, b_h1=None,
           w_z2=None, b_z2=None, w_r2=None, b_r2=None, w_h2=None, b_h2=None,
           w_lin1=None, b_lin1=None, w_lin2=None, b_lin2=None):
    in_maps = make_in_maps(x, w_z1, b_z1, w_r1, b_r1, w_h1, b_h1,
                           w_z2, b_z2, w_r2, b_r2, w_h2, b_h2,
                           w_lin1, b_lin1, w_lin2, b_lin2)
    nc = _get_nc()
    res = run_bass_kernel_spmd(nc, in_maps, list(range(N_CORES))).results
    n = np.asarray(x).shape[0]
    full = unscramble([res[i]["out"] for i in range(N_CORES)])
    return np.ascontiguousarray(full[:n].reshape(n, 1).astype(np.float32))


# revision 16
# speedup vs baseline: 1.2430x; 1.2430x over previous
"""Trainium2 Bass kernel for nn_EnhancedRecurrentGCN (K=1 DConv DCRNN stack).

Math (h0 == 0 collapses each DCRNN cell; the r-gate is multiplied by zero):
    h1 = relu(sigmoid(-x@W1z) * tanh(x@W1h))     [per node]
    h2 = relu(sigmoid(-h1@W2z) * tanh(h1@W2h))
    y  = relu(h2@W3 + b3) @ W4 + b4
with W1z = (w_z1[0,0]+w_z1[1,0])[:256] etc.  edge_index/edge_weight unused.

Design:
 - Pure data parallelism: x padded to 102400 rows, 12800 nodes/core, shipped
   pre-transposed and cast to fp16 (xt = x_shard.T) so the contraction dim is
   the SBUF partition dim.  All device tensors are feature-major.
 - fp16 matmul operands (fp32 PSUM accumulate): full PE rate, ~7e-4 rel err.
 - relu(s*u) = s*max(u,0) fused into one DVE scalar_tensor_tensor per layer.
 - Layer-2/3/4 outputs partition-packed via zero-embedded weight copies
   accumulating in psum (no tile_position, which fp32/fp16 dst checks reject):
   L2 packs 2 subtiles onto 64+64 partitions; L3 packs 8 subtiles onto
   16*8 partitions; L4 is ONE matmul per 8 subtiles via block-diag W4.
 - Two-stage software pipelining: macro m+1's L1 matmuls are emitted before
   macro m's tail so the in-order engine queues overlap across macros.
 - Single packed weight DMA + ramped x chunk sizes to shorten the prologue;
   per-group output DMAs to shorten the epilogue.
"""

import os
import sys

if "/opt/trn_rl_repo" not in sys.path:
    sys.path.insert(0, "/opt/trn_rl_repo")

from contextlib import ExitStack

import numpy as np
import ml_dtypes

import concourse.mybir as mybir
import concourse.tile as tile
from concourse import bacc
from concourse.bass_utils import run_bass_kernel_spmd

N_CORES = 8
PAD_NODES = 102400
SHARD = PAD_NODES // N_CORES  # 12800
TN = 512
CHUNK = 2560
GROUP = 8

F32 = mybir.dt.float32
F32R = mybir.dt.float32r
BF16 = mybir.dt.bfloat16
AF = mybir.ActivationFunctionType
OP = mybir.AluOpType

FP16 = mybir.dt.float16
_KMMDT = os.environ.get("KMMDT", "fp16")
MMDT = {"bf16": BF16, "fp16": FP16, "f32r": F32R}[_KMMDT]
NPDT = {"bf16": ml_dtypes.bfloat16, "fp16": np.float16, "f32r": np.float32}[_KMMDT]
# fp16 matmul operands: full PE rate (1 cycle/col vs 2 for fp32r), half the
# input DMA, and ~7e-4 relative error (all values here are << fp16 range).


def build_nc(shard=SHARD, mmdt=None):
    if mmdt is None:
        mmdt = MMDT
    assert shard % TN == 0
    n_sub = shard // TN
    groups = [list(range(g, min(g + GROUP, n_sub)))
              for g in range(0, n_sub, GROUP)]
    n_grp = len(groups)
    chunks = []
    c = 0
    ramp = [512, 1024, 2048]
    while c < shard:
        w = min(ramp[len(chunks)] if len(chunks) < len(ramp) else CHUNK,
                shard - c)
        chunks.append((c, w))
        c += w

    nc = bacc.Bacc(None)

    WCOLS = 2048 + GROUP  # 16 x 128-col weight slabs + the w4 embed
    xt = nc.declare_dram_parameter("xt", [256, shard], mmdt, isOutput=False)
    wpack = nc.declare_dram_parameter("wpack", [128, WCOLS], mmdt, isOutput=False)
    bpack = nc.declare_dram_parameter("bpack", [128, 6], F32, isOutput=False)
    out = nc.declare_dram_parameter("out", [GROUP, TN * n_grp], F32, isOutput=True)

    with ExitStack() as ctx:
        tc = ctx.enter_context(tile.TileContext(nc))
        wp = ctx.enter_context(tc.tile_pool(name="weights", bufs=1))
        xp = ctx.enter_context(tc.tile_pool(name="x", bufs=2))
        ap = ctx.enter_context(tc.tile_pool(name="acts", bufs=3))
        ob = ctx.enter_context(tc.tile_pool(name="outbuf", bufs=1))
        # PSUM: zpre 2 + tpre 2 + z2p 1 + t2p 1 + h3p 1 + opre 1 = 8 banks
        pz1 = ctx.enter_context(tc.tile_pool(name="pz1", bufs=1, space="PSUM"))
        pt1 = ctx.enter_context(tc.tile_pool(name="pt1", bufs=1, space="PSUM"))
        pz2 = ctx.enter_context(tc.tile_pool(name="pz2", bufs=1, space="PSUM"))
        pt2 = ctx.enter_context(tc.tile_pool(name="pt2", bufs=1, space="PSUM"))
        ph3 = ctx.enter_context(tc.tile_pool(name="ph3", bufs=1, space="PSUM"))
        po = ctx.enter_context(tc.tile_pool(name="po", bufs=1, space="PSUM"))

        wpack_sb = wp.tile([128, WCOLS], mmdt, name="wpack_sb")
        nc.sync.dma_start(wpack_sb[:], wpack[:])
        bpack_sb = wp.tile([128, 6], F32, name="bpack_sb")
        nc.sync.dma_start(bpack_sb[:], bpack[:])

        def wslab(k):
            return wpack_sb[:, 128 * k:128 * (k + 1)]

        w1z_sb = [wslab(0), wslab(1)]
        w1h_sb = [wslab(2), wslab(3)]
        w2ze_sb = [wslab(4), wslab(5)]
        w2he_sb = [wslab(6), wslab(7)]
        w3e_sb = [wslab(8 + j) for j in range(GROUP)]
        w4e_sb = wpack_sb[:, 2048:2048 + GROUP]
        bias_sb = {nm: bpack_sb[:, k:k + 1]
                   for k, nm in enumerate(["nbz1", "bh1", "nbz2p", "bh2p",
                                           "b3sp", "b4bc"])}

        out_sb = ob.tile([GROUP, TN * n_grp], F32)

        x_tiles = {}

        def ensure_chunk(ci):
            if ci in x_tiles or ci >= len(chunks):
                return
            c0, cw = chunks[ci]
            xa = xp.tile([128, cw], mmdt, tag="xa", name=f"xa{ci}")
            xb = xp.tile([128, cw], mmdt, tag="xb", name=f"xb{ci}")
            nc.sync.dma_start(xa[:], xt[0:128, c0:c0 + cw])
            nc.sync.dma_start(xb[:], xt[128:256, c0:c0 + cw])
            x_tiles[ci] = (xa, xb)

        def x_slice(s):
            col = s * TN
            ci = next(k for k, (c0, cw) in enumerate(chunks)
                      if c0 <= col < c0 + cw)
            ensure_chunk(ci)
            ensure_chunk(ci + 1)
            off = col - chunks[ci][0]
            return x_tiles[ci][0], x_tiles[ci][1], slice(off, off + TN)

        macros = [(2 * m, 2) for m in range(n_sub // 2)]
        if n_sub % 2:
            macros.append((n_sub - 1, 1))

        h3p_cur = [None]

        def stage_a(s0, nsub):
            """Layer-1 matmuls for macro (s0, nsub) -> (zpre, tpre)."""
            mw = nsub * TN
            zpre = pz1.tile([128, mw], F32, tag="zpre", name=f"zpre{s0}")
            tpre = pt1.tile([128, mw], F32, tag="tpre", name=f"tpre{s0}")
            for i in range(nsub):
                xa, xb, sl = x_slice(s0 + i)
                d = slice(i * TN, (i + 1) * TN)
                nc.tensor.matmul(zpre[:, d], w1z_sb[0], xa[:, sl],
                                 start=True, stop=False, skip_group_check=True)
                nc.tensor.matmul(zpre[:, d], w1z_sb[1], xb[:, sl],
                                 start=False, stop=True, skip_group_check=True)
            for i in range(nsub):
                xa, xb, sl = x_slice(s0 + i)
                d = slice(i * TN, (i + 1) * TN)
                nc.tensor.matmul(tpre[:, d], w1h_sb[0], xa[:, sl],
                                 start=True, stop=False, skip_group_check=True)
                nc.tensor.matmul(tpre[:, d], w1h_sb[1], xb[:, sl],
                                 start=False, stop=True, skip_group_check=True)
            return zpre, tpre

        def stage_b(s0, nsub, zpre, tpre):
            """ACT/DVE + layers 2-4 for macro (s0, nsub)."""
            mw = nsub * TN
            s1t = ap.tile([128, mw], F32, tag="s1t", name=f"s1t{s0}")
            nc.scalar.activation(s1t[:], zpre[:], AF.Sigmoid,
                                 bias=bias_sb["nbz1"], scale=-1.0)
            u1t = ap.tile([128, mw], F32, tag="u1t", name=f"u1t{s0}")
            nc.scalar.activation(u1t[:], tpre[:], AF.Tanh,
                                 bias=bias_sb["bh1"], scale=1.0)
            h1t = ap.tile([128, mw], mmdt, tag="h1t", name=f"h1t{s0}")
            nc.vector.scalar_tensor_tensor(h1t[:], u1t[:], 0.0, s1t[:],
                                           op0=OP.max, op1=OP.mult)

            z2p = pz2.tile([128, TN], F32, tag="z2p", name=f"z2p{s0}")
            t2p = pt2.tile([128, TN], F32, tag="t2p", name=f"t2p{s0}")
            for i in range(nsub):
                nc.tensor.matmul(z2p[:], w2ze_sb[i],
                                 h1t[:, i * TN:(i + 1) * TN],
                                 start=(i == 0), stop=(i == nsub - 1),
                                 skip_group_check=True)
            for i in range(nsub):
                nc.tensor.matmul(t2p[:], w2he_sb[i],
                                 h1t[:, i * TN:(i + 1) * TN],
                                 start=(i == 0), stop=(i == nsub - 1),
                                 skip_group_check=True)
            s2t = ap.tile([128, TN], F32, tag="s2t", name=f"s2t{s0}")
            nc.scalar.activation(s2t[:], z2p[:], AF.Sigmoid,
                                 bias=bias_sb["nbz2p"], scale=-1.0)
            u2t = ap.tile([128, TN], F32, tag="u2t", name=f"u2t{s0}")
            nc.scalar.activation(u2t[:], t2p[:], AF.Tanh,
                                 bias=bias_sb["bh2p"], scale=1.0)
            h2t = ap.tile([128, TN], mmdt, tag="h2t", name=f"h2t{s0}")
            nc.vector.scalar_tensor_tensor(h2t[:], u2t[:], 0.0, s2t[:],
                                           op0=OP.max, op1=OP.mult)

            for i in range(nsub):
                s = s0 + i
                g = s // GROUP
                j = s - groups[g][0]
                if j == 0:
                    h3p_cur[0] = ph3.tile([128, TN], F32, tag="h3p",
                                          name=f"h3p{g}")
                last = (j == len(groups[g]) - 1)
                nc.tensor.matmul(h3p_cur[0][:], w3e_sb[j], h2t[:],
                                 start=(j == 0), stop=last,
                                 skip_group_check=True)
                if last:
                    gs = len(groups[g])
                    h3s = ap.tile([128, TN], mmdt, tag="h3s", name=f"h3s{g}")
                    nc.vector.tensor_scalar(
                        h3s[0:16 * gs, :], h3p_cur[0][0:16 * gs, :],
                        bpack_sb[0:16 * gs, 4:5], 0.0,
                        op0=OP.add, op1=OP.max)
                    opre = po.tile([GROUP, TN], F32, tag="opre", name=f"op{g}")
                    nc.tensor.matmul(opre[0:gs, :], wpack_sb[0:16 * gs, 2048:2048 + gs],
                                     h3s[0:16 * gs, :], start=True, stop=True,
                                     skip_group_check=True)
                    nc.vector.tensor_scalar(
                        out_sb[0:gs, g * TN:(g + 1) * TN], opre[0:gs, :],
                        bpack_sb[0:gs, 5:6], None, op0=OP.add)
                    nc.sync.dma_start(out[0:gs, g * TN:(g + 1) * TN],
                                      out_sb[0:gs, g * TN:(g + 1) * TN])

        # two-stage software pipeline over macros
        pend = None
        for s0, nsub in macros:
            zp = stage_a(s0, nsub)
            if pend is not None:
                stage_b(*pend)
            pend = (s0, nsub, *zp)
        stage_b(*pend)

    nc.compile()
    return nc


_NC_CACHE = {}


def _get_nc(shard=SHARD):
    if shard not in _NC_CACHE:
        _NC_CACHE[shard] = build_nc(shard)
    return _NC_CACHE[shard]


def make_in_maps(x, w_z1, b_z1, w_r1, b_r1, w_h1, b_h1,
                 w_z2, b_z2, w_r2, b_r2, w_h2, b_h2,
                 w_lin1, b_lin1, w_lin2, b_lin2,
                 n_cores=N_CORES, shard=SHARD):
    f = np.float32
    w1z = np.asarray((np.asarray(w_z1)[0, 0] + np.asarray(w_z1)[1, 0])[:256], f)
    w1h = np.asarray((np.asarray(w_h1)[0, 0] + np.asarray(w_h1)[1, 0])[:256], f)
    w2z = np.asarray((np.asarray(w_z2)[0, 0] + np.asarray(w_z2)[1, 0])[:128], f)
    w2h = np.asarray((np.asarray(w_h2)[0, 0] + np.asarray(w_h2)[1, 0])[:128], f)
    w3 = np.asarray(w_lin1, f)
    w4 = np.asarray(w_lin2, f)

    wp = np.zeros((128, 2048 + GROUP), f)
    wp[:, 0:128] = w1z[0:128]
    wp[:, 128:256] = w1z[128:256]
    wp[:, 256:384] = w1h[0:128]
    wp[:, 384:512] = w1h[128:256]
    for v in range(2):
        wp[:, 512 + 128 * v + 64 * v:512 + 128 * v + 64 * v + 64] = w2z
        wp[:, 768 + 128 * v + 64 * v:768 + 128 * v + 64 * v + 64] = w2h
    for j in range(GROUP):
        h = 64 * (j % 2)
        wp[h:h + 64, 1024 + 128 * j + 16 * j:1024 + 128 * j + 16 * j + 16] = w3
    for j in range(GROUP):
        wp[16 * j:16 * j + 16, 2048 + j] = w4[:, 0]

    bp = np.zeros((128, 6), f)
    bp[:, 0] = -np.asarray(b_z1, f)
    bp[:, 1] = np.asarray(b_h1, f)
    bp[:, 2] = -np.tile(np.asarray(b_z2, f), 2)
    bp[:, 3] = np.tile(np.asarray(b_h2, f), 2)
    bp[:, 4] = np.tile(np.asarray(b_lin1, f), GROUP)
    bp[0:GROUP, 5] = np.asarray(b_lin2, f).reshape(-1)[0]
    common = {
        "wpack": wp.astype(NPDT),
        "bpack": bp,
    }
    x = np.asarray(x, f)
    n = x.shape[0]
    pad = n_cores * shard
    xpad = np.zeros((pad, 256), f)
    xpad[:n] = x
    shards = xpad.reshape(n_cores, shard, 256)
    return [dict(common, xt=np.ascontiguousarray(shards[i].T).astype(NPDT))
            for i in range(n_cores)]


def unscramble(res, n_cores=N_CORES, shard=SHARD):
    n_sub = shard // TN
    full = np.empty(n_cores * shard, np.float32)
    for i in range(n_cores):
        o = res[i]
        for g in range((n_sub + GROUP - 1) // GROUP):
            gs = min(GROUP, n_sub - g * GROUP)
            for j in range(gs):
                s = g * GROUP + j
                full[i * shard + s * TN:i * shard + (s + 1) * TN] = \
                    o[j, g * TN:(g + 1) * TN]
    return full


def kernel(x, edge_index=None, edge_weight=None,
           w_z1=None, b_z1=None, w_r1=None, b_r1=None, w_h1=None, b_h1=None,
           w_z2=None, b_z2=None, w_r2=None, b_r2=None, w_h2=None, b_h2=None,
           w_lin1=None, b_lin1=None, w_lin2=None, b_lin2=None):
    in_maps = make_in_maps(x, w_z1, b_z1, w_r1, b_r1, w_h1, b_h1,
                           w_z2, b_z2, w_r2, b_r2, w_h2, b_h2,
                           w_lin1, b_lin1, w_lin2, b_lin2)
    nc = _get_nc()
    res = run_bass_kernel_spmd(nc, in_maps, list(range(N_CORES))).results
    n = np.asarray(x).shape[0]
    full = unscramble([res[i]["out"] for i in range(N_CORES)])
    return np.ascontiguousarray(full[:n].reshape(n, 1).astype(np.float32))


# revision 17
# speedup vs baseline: 1.2574x; 1.0116x over previous
"""Trainium2 Bass kernel for nn_EnhancedRecurrentGCN (K=1 DConv DCRNN stack).

Math (h0 == 0 collapses each DCRNN cell; the r-gate is multiplied by zero):
    h1 = relu(sigmoid(-x@W1z) * tanh(x@W1h))     [per node]
    h2 = relu(sigmoid(-h1@W2z) * tanh(h1@W2h))
    y  = relu(h2@W3 + b3) @ W4 + b4
with W1z = (w_z1[0,0]+w_z1[1,0])[:256] etc.  edge_index/edge_weight unused.

Design:
 - Pure data parallelism: x padded to 102400 rows, 12800 nodes/core, shipped
   pre-transposed and cast to fp16 (xt = x_shard.T) so the contraction dim is
   the SBUF partition dim.  All device tensors are feature-major.
 - fp16 matmul operands (fp32 PSUM accumulate): full PE rate, ~7e-4 rel err.
 - relu(s*u) = s*max(u,0) fused into one DVE scalar_tensor_tensor per layer.
 - Layer-2/3/4 outputs partition-packed via zero-embedded weight copies
   accumulating in psum (no tile_position, which fp32/fp16 dst checks reject):
   L2 packs 2 subtiles onto 64+64 partitions; L3 packs 8 subtiles onto
   16*8 partitions; L4 is ONE matmul per 8 subtiles via block-diag W4.
 - Two-stage software pipelining: macro m+1's L1 matmuls are emitted before
   macro m's tail so the in-order engine queues overlap across macros.
 - Single packed weight DMA + ramped x chunk sizes to shorten the prologue;
   per-group output DMAs to shorten the epilogue.
"""

import os
import sys

if "/opt/trn_rl_repo" not in sys.path:
    sys.path.insert(0, "/opt/trn_rl_repo")

from contextlib import ExitStack

import numpy as np
import ml_dtypes

import concourse.mybir as mybir
import concourse.tile as tile
from concourse import bacc
from concourse.bass_utils import run_bass_kernel_spmd

N_CORES = 8
PAD_NODES = 102400
SHARD = PAD_NODES // N_CORES  # 12800
TN = 512
CHUNK = 2560
GROUP = 8

F32 = mybir.dt.float32
F32R = mybir.dt.float32r
BF16 = mybir.dt.bfloat16
AF = mybir.ActivationFunctionType
OP = mybir.AluOpType

FP16 = mybir.dt.float16
_KMMDT = os.environ.get("KMMDT", "fp16")
MMDT = {"bf16": BF16, "fp16": FP16, "f32r": F32R}[_KMMDT]
NPDT = {"bf16": ml_dtypes.bfloat16, "fp16": np.float16, "f32r": np.float32}[_KMMDT]
# fp16 matmul operands: full PE rate (1 cycle/col vs 2 for fp32r), half the
# input DMA, and ~7e-4 relative error (all values here are << fp16 range).


def build_nc(shard=SHARD, mmdt=None):
    if mmdt is None:
        mmdt = MMDT
    assert shard % TN == 0
    n_sub = shard // TN
    groups = [list(range(g, min(g + GROUP, n_sub)))
              for g in range(0, n_sub, GROUP)]
    n_grp = len(groups)
    chunks = []
    c = 0
    ramp = [512, 512, 1024, 2048]
    while c < shard:
        w = min(ramp[len(chunks)] if len(chunks) < len(ramp) else CHUNK,
                shard - c)
        chunks.append((c, w))
        c += w

    nc = bacc.Bacc(None)

    WCOLS = 2048 + GROUP  # 16 x 128-col weight slabs + the w4 embed
    xt = nc.declare_dram_parameter("xt", [256, shard], mmdt, isOutput=False)
    wpack = nc.declare_dram_parameter("wpack", [128, WCOLS], mmdt, isOutput=False)
    bpack = nc.declare_dram_parameter("bpack", [128, 6], F32, isOutput=False)
    out = nc.declare_dram_parameter("out", [GROUP, TN * n_grp], F32, isOutput=True)

    with ExitStack() as ctx:
        tc = ctx.enter_context(tile.TileContext(nc))
        wp = ctx.enter_context(tc.tile_pool(name="weights", bufs=1))
        xp = ctx.enter_context(tc.tile_pool(name="x", bufs=3))
        ap = ctx.enter_context(tc.tile_pool(name="acts", bufs=4))
        ob = ctx.enter_context(tc.tile_pool(name="outbuf", bufs=1))
        # PSUM: zpre 2 + tpre 2 + z2p 1 + t2p 1 + h3p 1 + opre 1 = 8 banks
        pz1 = ctx.enter_context(tc.tile_pool(name="pz1", bufs=1, space="PSUM"))
        pt1 = ctx.enter_context(tc.tile_pool(name="pt1", bufs=1, space="PSUM"))
        pz2 = ctx.enter_context(tc.tile_pool(name="pz2", bufs=1, space="PSUM"))
        pt2 = ctx.enter_context(tc.tile_pool(name="pt2", bufs=1, space="PSUM"))
        ph3 = ctx.enter_context(tc.tile_pool(name="ph3", bufs=1, space="PSUM"))
        po = ctx.enter_context(tc.tile_pool(name="po", bufs=1, space="PSUM"))

        wpack_sb = wp.tile([128, WCOLS], mmdt, name="wpack_sb")
        nc.sync.dma_start(wpack_sb[:], wpack[:])
        bpack_sb = wp.tile([128, 6], F32, name="bpack_sb")
        nc.sync.dma_start(bpack_sb[:], bpack[:])

        def wslab(k):
            return wpack_sb[:, 128 * k:128 * (k + 1)]

        w1z_sb = [wslab(0), wslab(1)]
        w1h_sb = [wslab(2), wslab(3)]
        w2ze_sb = [wslab(4), wslab(5)]
        w2he_sb = [wslab(6), wslab(7)]
        w3e_sb = [wslab(8 + j) for j in range(GROUP)]
        w4e_sb = wpack_sb[:, 2048:2048 + GROUP]
        bias_sb = {nm: bpack_sb[:, k:k + 1]
                   for k, nm in enumerate(["nbz1", "bh1", "nbz2p", "bh2p",
                                           "b3sp", "b4bc"])}

        out_sb = ob.tile([GROUP, TN * n_grp], F32)

        x_tiles = {}

        def ensure_chunk(ci):
            if ci in x_tiles or ci >= len(chunks):
                return
            c0, cw = chunks[ci]
            xa = xp.tile([128, cw], mmdt, tag="xa", name=f"xa{ci}")
            xb = xp.tile([128, cw], mmdt, tag="xb", name=f"xb{ci}")
            nc.sync.dma_start(xa[:], xt[0:128, c0:c0 + cw])
            nc.sync.dma_start(xb[:], xt[128:256, c0:c0 + cw])
            x_tiles[ci] = (xa, xb)

        def x_slice(s):
            col = s * TN
            ci = next(k for k, (c0, cw) in enumerate(chunks)
                      if c0 <= col < c0 + cw)
            ensure_chunk(ci)
            ensure_chunk(ci + 1)
            off = col - chunks[ci][0]
            return x_tiles[ci][0], x_tiles[ci][1], slice(off, off + TN)

        macros = [(2 * m, 2) for m in range(n_sub // 2)]
        if n_sub % 2:
            macros.append((n_sub - 1, 1))

        h3p_cur = [None]

        def stage_a(s0, nsub):
            """Layer-1 matmuls for macro (s0, nsub) -> (zpre, tpre)."""
            mw = nsub * TN
            zpre = pz1.tile([128, mw], F32, tag="zpre", name=f"zpre{s0}")
            tpre = pt1.tile([128, mw], F32, tag="tpre", name=f"tpre{s0}")
            for i in range(nsub):
                xa, xb, sl = x_slice(s0 + i)
                d = slice(i * TN, (i + 1) * TN)
                nc.tensor.matmul(zpre[:, d], w1z_sb[0], xa[:, sl],
                                 start=True, stop=False, skip_group_check=True)
                nc.tensor.matmul(zpre[:, d], w1z_sb[1], xb[:, sl],
                                 start=False, stop=True, skip_group_check=True)
            for i in range(nsub):
                xa, xb, sl = x_slice(s0 + i)
                d = slice(i * TN, (i + 1) * TN)
                nc.tensor.matmul(tpre[:, d], w1h_sb[0], xa[:, sl],
                                 start=True, stop=False, skip_group_check=True)
                nc.tensor.matmul(tpre[:, d], w1h_sb[1], xb[:, sl],
                                 start=False, stop=True, skip_group_check=True)
            return zpre, tpre

        def stage_b(s0, nsub, zpre, tpre):
            """ACT/DVE + layers 2-4 for macro (s0, nsub)."""
            mw = nsub * TN
            s1t = ap.tile([128, mw], F32, tag="s1t", name=f"s1t{s0}")
            nc.scalar.activation(s1t[:], zpre[:], AF.Sigmoid,
                                 bias=bias_sb["nbz1"], scale=-1.0)
            u1t = ap.tile([128, mw], F32, tag="u1t", name=f"u1t{s0}")
            nc.scalar.activation(u1t[:], tpre[:], AF.Tanh,
                                 bias=bias_sb["bh1"], scale=1.0)
            h1t = ap.tile([128, mw], mmdt, tag="h1t", name=f"h1t{s0}")
            nc.vector.scalar_tensor_tensor(h1t[:], u1t[:], 0.0, s1t[:],
                                           op0=OP.max, op1=OP.mult)

            z2p = pz2.tile([128, TN], F32, tag="z2p", name=f"z2p{s0}")
            t2p = pt2.tile([128, TN], F32, tag="t2p", name=f"t2p{s0}")
            for i in range(nsub):
                nc.tensor.matmul(z2p[:], w2ze_sb[i],
                                 h1t[:, i * TN:(i + 1) * TN],
                                 start=(i == 0), stop=(i == nsub - 1),
                                 skip_group_check=True)
            for i in range(nsub):
                nc.tensor.matmul(t2p[:], w2he_sb[i],
                                 h1t[:, i * TN:(i + 1) * TN],
                                 start=(i == 0), stop=(i == nsub - 1),
                                 skip_group_check=True)
            s2t = ap.tile([128, TN], F32, tag="s2t", name=f"s2t{s0}")
            nc.scalar.activation(s2t[:], z2p[:], AF.Sigmoid,
                                 bias=bias_sb["nbz2p"], scale=-1.0)
            u2t = ap.tile([128, TN], F32, tag="u2t", name=f"u2t{s0}")
            nc.scalar.activation(u2t[:], t2p[:], AF.Tanh,
                                 bias=bias_sb["bh2p"], scale=1.0)
            h2t = ap.tile([128, TN], mmdt, tag="h2t", name=f"h2t{s0}")
            nc.vector.scalar_tensor_tensor(h2t[:], u2t[:], 0.0, s2t[:],
                                           op0=OP.max, op1=OP.mult)

            for i in range(nsub):
                s = s0 + i
                g = s // GROUP
                j = s - groups[g][0]
                if j == 0:
                    h3p_cur[0] = ph3.tile([128, TN], F32, tag="h3p",
                                          name=f"h3p{g}")
                last = (j == len(groups[g]) - 1)
                nc.tensor.matmul(h3p_cur[0][:], w3e_sb[j], h2t[:],
                                 start=(j == 0), stop=last,
                                 skip_group_check=True)
                if last:
                    gs = len(groups[g])
                    h3s = ap.tile([128, TN], mmdt, tag="h3s", name=f"h3s{g}")
                    nc.vector.tensor_scalar(
                        h3s[0:16 * gs, :], h3p_cur[0][0:16 * gs, :],
                        bpack_sb[0:16 * gs, 4:5], 0.0,
                        op0=OP.add, op1=OP.max)
                    opre = po.tile([GROUP, TN], F32, tag="opre", name=f"op{g}")
                    nc.tensor.matmul(opre[0:gs, :], wpack_sb[0:16 * gs, 2048:2048 + gs],
                                     h3s[0:16 * gs, :], start=True, stop=True,
                                     skip_group_check=True)
                    nc.vector.tensor_scalar(
                        out_sb[0:gs, g * TN:(g + 1) * TN], opre[0:gs, :],
                        bpack_sb[0:gs, 5:6], None, op0=OP.add)
                    nc.sync.dma_start(out[0:gs, g * TN:(g + 1) * TN],
                                      out_sb[0:gs, g * TN:(g + 1) * TN])

        # two-stage software pipeline over macros
        pend = None
        for s0, nsub in macros:
            zp = stage_a(s0, nsub)
            if pend is not None:
                stage_b(*pend)
            pend = (s0, nsub, *zp)
        stage_b(*pend)

    nc.compile()
    return nc


_NC_CACHE = {}


def _get_nc(shard=SHARD):
    if shard not in _NC_CACHE:
        _NC_CACHE[shard] = build_nc(shard)
    return _NC_CACHE[shard]


def make_in_maps(x, w_z1, b_z1, w_r1, b_r1, w_h1, b_h1,
                 w_z2, b_z2, w_r2, b_r2, w_h2, b_h2,
                 w_lin1, b_lin1, w_lin2, b_lin2,
                 n_cores=N_CORES, shard=SHARD):
    f = np.float32
    w1z = np.asarray((np.asarray(w_z1)[0, 0] + np.asarray(w_z1)[1, 0])[:256], f)
    w1h = np.asarray((np.asarray(w_h1)[0, 0] + np.asarray(w_h1)[1, 0])[:256], f)
    w2z = np.asarray((np.asarray(w_z2)[0, 0] + np.asarray(w_z2)[1, 0])[:128], f)
    w2h = np.asarray((np.asarray(w_h2)[0, 0] + np.asarray(w_h2)[1, 0])[:128], f)
    w3 = np.asarray(w_lin1, f)
    w4 = np.asarray(w_lin2, f)

    wp = np.zeros((128, 2048 + GROUP), f)
    wp[:, 0:128] = w1z[0:128]
    wp[:, 128:256] = w1z[128:256]
    wp[:, 256:384] = w1h[0:128]
    wp[:, 384:512] = w1h[128:256]
    for v in range(2):
        wp[:, 512 + 128 * v + 64 * v:512 + 128 * v + 64 * v + 64] = w2z
        wp[:, 768 + 128 * v + 64 * v:768 + 128 * v + 64 * v + 64] = w2h
    for j in range(GROUP):
        h = 64 * (j % 2)
        wp[h:h + 64, 1024 + 128 * j + 16 * j:1024 + 128 * j + 16 * j + 16] = w3
    for j in range(GROUP):
        wp[16 * j:16 * j + 16, 2048 + j] = w4[:, 0]

    bp = np.zeros((128, 6), f)
    bp[:, 0] = -np.asarray(b_z1, f)
    bp[:, 1] = np.asarray(b_h1, f)
    bp[:, 2] = -np.tile(np.asarray(b_z2, f), 2)
    bp[:, 3] = np.tile(np.asarray(b_h2, f), 2)
    bp[:, 4] = np.tile(np.asarray(b_lin1, f), GROUP)
    bp[0:GROUP, 5] = np.asarray(b_lin2, f).reshape(-1)[0]
    common = {
        "wpack": wp.astype(NPDT),
        "bpack": bp,
    }
    x = np.asarray(x, f)
    n = x.shape[0]
    pad = n_cores * shard
    xpad = np.zeros((pad, 256), f)
    xpad[:n] = x
    shards = xpad.reshape(n_cores, shard, 256)
    return [dict(common, xt=np.ascontiguousarray(shards[i].T).astype(NPDT))
            for i in range(n_cores)]


def unscramble(res, n_cores=N_CORES, shard=SHARD):
    n_sub = shard // TN
    full = np.empty(n_cores * shard, np.float32)
    for i in range(n_cores):
        o = res[i]
        for g in range((n_sub + GROUP - 1) // GROUP):
            gs = min(GROUP, n_sub - g * GROUP)
            for j in range(gs):
                s = g * GROUP + j
                full[i * shard + s * TN:i * shard + (s + 1) * TN] = \
                    o[j, g * TN:(g + 1) * TN]
    return full


def kernel(x, edge_index=None, edge_weight=None,
           w_z1=None, b_z1=None, w_r1=None, b_r1=None, w_h1=None, b_h1=None,
           w_z2=None, b_z2=None, w_r2=None, b_r2=None, w_h2=None, b_h2=None,
           w_lin1=None, b_lin1=None, w_lin2=None, b_lin2=None):
    in_maps = make_in_maps(x, w_z1, b_z1, w_r1, b_r1, w_h1, b_h1,
                           w_z2, b_z2, w_r2, b_r2, w_h2, b_h2,
                           w_lin1, b_lin1, w_lin2, b_lin2)
    nc = _get_nc()
    res = run_bass_kernel_spmd(nc, in_maps, list(range(N_CORES))).results
    n = np.asarray(x).shape[0]
    full = unscramble([res[i]["out"] for i in range(N_CORES)])
    return np.ascontiguousarray(full[:n].reshape(n, 1).astype(np.float32))


# revision 18
# speedup vs baseline: 1.2813x; 1.0190x over previous
"""Trainium2 Bass kernel for nn_EnhancedRecurrentGCN (K=1 DConv DCRNN stack).

Math (h0 == 0 collapses each DCRNN cell; the r-gate is multiplied by zero):
    h1 = relu(sigmoid(-x@W1z) * tanh(x@W1h))     [per node]
    h2 = relu(sigmoid(-h1@W2z) * tanh(h1@W2h))
    y  = relu(h2@W3 + b3) @ W4 + b4
with W1z = (w_z1[0,0]+w_z1[1,0])[:256] etc.  edge_index/edge_weight unused.

Design:
 - Pure data parallelism: x padded to 102400 rows, 12800 nodes/core, shipped
   pre-transposed and cast to fp16 (xt = x_shard.T) so the contraction dim is
   the SBUF partition dim.  All device tensors are feature-major.
 - fp16 matmul operands (fp32 PSUM accumulate): full PE rate, ~9e-4 rel err.
 - relu(s*u) = s*max(u,0) fused into one DVE scalar_tensor_tensor per layer.
 - Layer-2/3/4 outputs partition-packed via zero-embedded weight copies
   accumulating in psum (tile_position col/row offsets are rejected for
   these dtypes): L2 packs 2 subtiles onto 64+64 partitions; L3 packs 8
   subtiles onto 16*8 partitions; L4 is ONE matmul per 8 subtiles.
 - Layer-2 sigma fusion: tanh(b) = 2*sigmoid(2b)-1 with the -2 baked into
   the zero-embedded W2h copies, so sigmoid+tanh is one ACT call per macro.
 - Two-stage software pipelining (macro m+1's L1 matmuls emitted before
   macro m's tail) to overlap the in-order engine queues.
 - Split packed weight DMA + ramped x chunks to shorten the prologue;
   per-group output DMAs to shorten the epilogue.
"""

import os
import sys

if "/opt/trn_rl_repo" not in sys.path:
    sys.path.insert(0, "/opt/trn_rl_repo")

from contextlib import ExitStack

import numpy as np
import ml_dtypes

import concourse.mybir as mybir
import concourse.tile as tile
from concourse import bacc
from concourse.bass_utils import run_bass_kernel_spmd

N_CORES = 8
PAD_NODES = 102400
SHARD = PAD_NODES // N_CORES  # 12800
TN = 512
CHUNK = 2560
GROUP = 8

F32 = mybir.dt.float32
F32R = mybir.dt.float32r
BF16 = mybir.dt.bfloat16
AF = mybir.ActivationFunctionType
OP = mybir.AluOpType

FP16 = mybir.dt.float16
_KMMDT = os.environ.get("KMMDT", "fp16")
MMDT = {"bf16": BF16, "fp16": FP16, "f32r": F32R}[_KMMDT]
NPDT = {"bf16": ml_dtypes.bfloat16, "fp16": np.float16, "f32r": np.float32}[_KMMDT]
# fp16 matmul operands: full PE rate (1 cycle/col vs 2 for fp32r), half the
# input DMA, and ~7e-4 relative error (all values here are << fp16 range).


def build_nc(shard=SHARD, mmdt=None):
    if mmdt is None:
        mmdt = MMDT
    assert shard % TN == 0
    n_sub = shard // TN
    groups = [list(range(g, min(g + GROUP, n_sub)))
              for g in range(0, n_sub, GROUP)]
    n_grp = len(groups)
    chunks = []
    c = 0
    ramp = [512, 512, 1024, 2048]
    while c < shard:
        w = min(ramp[len(chunks)] if len(chunks) < len(ramp) else CHUNK,
                shard - c)
        chunks.append((c, w))
        c += w

    nc = bacc.Bacc(None)

    WCOLS = 2048 + GROUP  # 16 x 128-col weight slabs + the w4 embed
    xt = nc.declare_dram_parameter("xt", [256, shard], mmdt, isOutput=False)
    wpackA = nc.declare_dram_parameter("wpackA", [128, 512], mmdt, isOutput=False)
    wpackB = nc.declare_dram_parameter("wpackB", [128, WCOLS - 512], mmdt,
                                       isOutput=False)
    bpack = nc.declare_dram_parameter("bpack", [128, 6], F32, isOutput=False)
    out = nc.declare_dram_parameter("out", [GROUP, TN * n_grp], F32, isOutput=True)

    with ExitStack() as ctx:
        tc = ctx.enter_context(tile.TileContext(nc))
        wp = ctx.enter_context(tc.tile_pool(name="weights", bufs=1))
        xp = ctx.enter_context(tc.tile_pool(name="x", bufs=3))
        ap = ctx.enter_context(tc.tile_pool(name="acts", bufs=4))
        ob = ctx.enter_context(tc.tile_pool(name="outbuf", bufs=1))
        # PSUM: zpre 2 + tpre 2 + z2p 1 + t2p 1 + h3p 1 + opre 1 = 8 banks
        pz1 = ctx.enter_context(tc.tile_pool(name="pz1", bufs=1, space="PSUM"))
        pt1 = ctx.enter_context(tc.tile_pool(name="pt1", bufs=1, space="PSUM"))
        pzt2 = ctx.enter_context(tc.tile_pool(name="pzt2", bufs=1, space="PSUM"))
        ph3 = ctx.enter_context(tc.tile_pool(name="ph3", bufs=1, space="PSUM"))
        po = ctx.enter_context(tc.tile_pool(name="po", bufs=1, space="PSUM"))

        wpack_sb = wp.tile([128, WCOLS], mmdt, name="wpack_sb")
        nc.sync.dma_start(wpack_sb[:, 0:512], wpackA[:])
        nc.sync.dma_start(wpack_sb[:, 512:WCOLS], wpackB[:])
        bpack_sb = wp.tile([128, 6], F32, name="bpack_sb")
        nc.sync.dma_start(bpack_sb[:], bpack[:])

        def wslab(k):
            return wpack_sb[:, 128 * k:128 * (k + 1)]

        w1z_sb = [wslab(0), wslab(1)]
        w1h_sb = [wslab(2), wslab(3)]
        w2ze_sb = [wslab(4), wslab(5)]
        w2he_sb = [wslab(6), wslab(7)]
        w3e_sb = [wslab(8 + j) for j in range(GROUP)]
        w4e_sb = wpack_sb[:, 2048:2048 + GROUP]
        bias_sb = {nm: bpack_sb[:, k:k + 1]
                   for k, nm in enumerate(["nbz1", "bh1", "nbz2p", "bh2p",
                                           "b3sp", "b4bc"])}

        out_sb = ob.tile([GROUP, TN * n_grp], F32)

        x_tiles = {}

        def ensure_chunk(ci):
            if ci in x_tiles or ci >= len(chunks):
                return
            c0, cw = chunks[ci]
            xa = xp.tile([128, cw], mmdt, tag="xa", name=f"xa{ci}")
            xb = xp.tile([128, cw], mmdt, tag="xb", name=f"xb{ci}")
            nc.sync.dma_start(xa[:], xt[0:128, c0:c0 + cw])
            nc.sync.dma_start(xb[:], xt[128:256, c0:c0 + cw])
            x_tiles[ci] = (xa, xb)

        def x_slice(s):
            col = s * TN
            ci = next(k for k, (c0, cw) in enumerate(chunks)
                      if c0 <= col < c0 + cw)
            ensure_chunk(ci)
            ensure_chunk(ci + 1)
            off = col - chunks[ci][0]
            return x_tiles[ci][0], x_tiles[ci][1], slice(off, off + TN)

        macros = [(2 * m, 2) for m in range(n_sub // 2)]
        if n_sub % 2:
            macros.append((n_sub - 1, 1))

        h3p_cur = [None]

        def stage_a(s0, nsub):
            """Layer-1 matmuls for macro (s0, nsub) -> (zpre, tpre)."""
            mw = nsub * TN
            zpre = pz1.tile([128, mw], F32, tag="zpre", name=f"zpre{s0}")
            tpre = pt1.tile([128, mw], F32, tag="tpre", name=f"tpre{s0}")
            for i in range(nsub):
                xa, xb, sl = x_slice(s0 + i)
                d = slice(i * TN, (i + 1) * TN)
                nc.tensor.matmul(zpre[:, d], w1z_sb[0], xa[:, sl],
                                 start=True, stop=False, skip_group_check=True)
                nc.tensor.matmul(zpre[:, d], w1z_sb[1], xb[:, sl],
                                 start=False, stop=True, skip_group_check=True)
            for i in range(nsub):
                xa, xb, sl = x_slice(s0 + i)
                d = slice(i * TN, (i + 1) * TN)
                nc.tensor.matmul(tpre[:, d], w1h_sb[0], xa[:, sl],
                                 start=True, stop=False, skip_group_check=True)
                nc.tensor.matmul(tpre[:, d], w1h_sb[1], xb[:, sl],
                                 start=False, stop=True, skip_group_check=True)
            return zpre, tpre

        def stage_b(s0, nsub, zpre, tpre):
            """ACT/DVE + layers 2-4 for macro (s0, nsub)."""
            mw = nsub * TN
            s1t = ap.tile([128, mw], F32, tag="s1t", name=f"s1t{s0}")
            nc.scalar.activation(s1t[:], zpre[:], AF.Sigmoid,
                                 bias=bias_sb["nbz1"], scale=-1.0)
            u1t = ap.tile([128, mw], F32, tag="u1t", name=f"u1t{s0}")
            nc.scalar.activation(u1t[:], tpre[:], AF.Tanh,
                                 bias=bias_sb["bh1"], scale=1.0)
            h1t = ap.tile([128, mw], mmdt, tag="h1t", name=f"h1t{s0}")
            nc.vector.scalar_tensor_tensor(h1t[:], u1t[:], 0.0, s1t[:],
                                           op0=OP.max, op1=OP.mult)

            # zt2 = [ a2 | -2*b2 ]; one sigmoid(-x) gives [ s2 | sig(2b2) ]
            # and tanh(b2) = 2*sig(2b2) - 1  (w2he embeds carry the -2).
            zt2 = pzt2.tile([128, 2 * TN], F32, tag="zt2", name=f"zt2{s0}")
            for half, wsb in enumerate([w2ze_sb, w2he_sb]):
                d = slice(half * TN, (half + 1) * TN)
                for i in range(nsub):
                    nc.tensor.matmul(zt2[:, d], wsb[i],
                                     h1t[:, i * TN:(i + 1) * TN],
                                     start=(i == 0), stop=(i == nsub - 1),
                                     skip_group_check=True)
            sv2 = ap.tile([128, 2 * TN], F32, tag="sv2", name=f"sv2{s0}")
            nc.scalar.activation(sv2[:], zt2[:], AF.Sigmoid, scale=-1.0)
            w2i = ap.tile([128, TN], F32, tag="w2i", name=f"w2i{s0}")
            nc.vector.scalar_tensor_tensor(w2i[:], sv2[:, TN:2 * TN], 0.5,
                                           sv2[:, 0:TN], op0=OP.max,
                                           op1=OP.mult)
            h2t = ap.tile([128, TN], mmdt, tag="h2t", name=f"h2t{s0}")
            nc.vector.scalar_tensor_tensor(h2t[:], w2i[:], 2.0, sv2[:, 0:TN],
                                           op0=OP.mult, op1=OP.subtract)

            for i in range(nsub):
                s = s0 + i
                g = s // GROUP
                j = s - groups[g][0]
                if j == 0:
                    h3p_cur[0] = ph3.tile([128, TN], F32, tag="h3p",
                                          name=f"h3p{g}")
                last = (j == len(groups[g]) - 1)
                nc.tensor.matmul(h3p_cur[0][:], w3e_sb[j], h2t[:],
                                 start=(j == 0), stop=last,
                                 skip_group_check=True)
                if last:
                    gs = len(groups[g])
                    h3s = ap.tile([128, TN], mmdt, tag="h3s", name=f"h3s{g}")
                    nc.vector.tensor_scalar(
                        h3s[0:16 * gs, :], h3p_cur[0][0:16 * gs, :],
                        bpack_sb[0:16 * gs, 4:5], 0.0,
                        op0=OP.add, op1=OP.max)
                    opre = po.tile([GROUP, TN], F32, tag="opre", name=f"op{g}")
                    nc.tensor.matmul(opre[0:gs, :], wpack_sb[0:16 * gs, 2048:2048 + gs],
                                     h3s[0:16 * gs, :], start=True, stop=True,
                                     skip_group_check=True)
                    nc.vector.tensor_scalar(
                        out_sb[0:gs, g * TN:(g + 1) * TN], opre[0:gs, :],
                        bpack_sb[0:gs, 5:6], None, op0=OP.add)
                    nc.sync.dma_start(out[0:gs, g * TN:(g + 1) * TN],
                                      out_sb[0:gs, g * TN:(g + 1) * TN])

        # two-stage software pipeline over macros
        pend = None
        for s0, nsub in macros:
            zp = stage_a(s0, nsub)
            if pend is not None:
                stage_b(*pend)
            pend = (s0, nsub, *zp)
        stage_b(*pend)

    nc.compile()
    return nc


_NC_CACHE = {}


def _get_nc(shard=SHARD):
    if shard not in _NC_CACHE:
        _NC_CACHE[shard] = build_nc(shard)
    return _NC_CACHE[shard]


def make_in_maps(x, w_z1, b_z1, w_r1, b_r1, w_h1, b_h1,
                 w_z2, b_z2, w_r2, b_r2, w_h2, b_h2,
                 w_lin1, b_lin1, w_lin2, b_lin2,
                 n_cores=N_CORES, shard=SHARD):
    f = np.float32
    for b in (b_z2, b_h2):
        assert not np.any(np.asarray(b)), \
            "sigma-fused layer 2 assumes zero gate biases (spec: fill=zeros)"
    w1z = np.asarray((np.asarray(w_z1)[0, 0] + np.asarray(w_z1)[1, 0])[:256], f)
    w1h = np.asarray((np.asarray(w_h1)[0, 0] + np.asarray(w_h1)[1, 0])[:256], f)
    w2z = np.asarray((np.asarray(w_z2)[0, 0] + np.asarray(w_z2)[1, 0])[:128], f)
    w2h = np.asarray((np.asarray(w_h2)[0, 0] + np.asarray(w_h2)[1, 0])[:128], f)
    w3 = np.asarray(w_lin1, f)
    w4 = np.asarray(w_lin2, f)

    wp = np.zeros((128, 2048 + GROUP), f)
    wp[:, 0:128] = w1z[0:128]
    wp[:, 128:256] = w1z[128:256]
    wp[:, 256:384] = w1h[0:128]
    wp[:, 384:512] = w1h[128:256]
    for v in range(2):
        wp[:, 512 + 128 * v + 64 * v:512 + 128 * v + 64 * v + 64] = w2z
        wp[:, 768 + 128 * v + 64 * v:768 + 128 * v + 64 * v + 64] = -2.0 * w2h
    for j in range(GROUP):
        h = 64 * (j % 2)
        wp[h:h + 64, 1024 + 128 * j + 16 * j:1024 + 128 * j + 16 * j + 16] = w3
    for j in range(GROUP):
        wp[16 * j:16 * j + 16, 2048 + j] = w4[:, 0]

    bp = np.zeros((128, 6), f)
    bp[:, 0] = -np.asarray(b_z1, f)
    bp[:, 1] = np.asarray(b_h1, f)
    bp[:, 2] = -np.tile(np.asarray(b_z2, f), 2)
    bp[:, 3] = np.tile(np.asarray(b_h2, f), 2)
    bp[:, 4] = np.tile(np.asarray(b_lin1, f), GROUP)
    bp[0:GROUP, 5] = np.asarray(b_lin2, f).reshape(-1)[0]
    wpn = wp.astype(NPDT)
    common = {
        "wpackA": np.ascontiguousarray(wpn[:, 0:512]),
        "wpackB": np.ascontiguousarray(wpn[:, 512:]),
        "bpack": bp,
    }
    x = np.asarray(x, f)
    n = x.shape[0]
    pad = n_cores * shard
    xpad = np.zeros((pad, 256), f)
    xpad[:n] = x
    shards = xpad.reshape(n_cores, shard, 256)
    return [dict(common, xt=np.ascontiguousarray(shards[i].T).astype(NPDT))
            for i in range(n_cores)]


def unscramble(res, n_cores=N_CORES, shard=SHARD):
    n_sub = shard // TN
    full = np.empty(n_cores * shard, np.float32)
    for i in range(n_cores):
        o = res[i]
        for g in range((n_sub + GROUP - 1) // GROUP):
            gs = min(GROUP, n_sub - g * GROUP)
            for j in range(gs):
                s = g * GROUP + j
                full[i * shard + s * TN:i * shard + (s + 1) * TN] = \
                    o[j, g * TN:(g + 1) * TN]
    return full


def kernel(x, edge_index=None, edge_weight=None,
           w_z1=None, b_z1=None, w_r1=None, b_r1=None, w_h1=None, b_h1=None,
           w_z2=None, b_z2=None, w_r2=None, b_r2=None, w_h2=None, b_h2=None,
           w_lin1=None, b_lin1=None, w_lin2=None, b_lin2=None):
    in_maps = make_in_maps(x, w_z1, b_z1, w_r1, b_r1, w_h1, b_h1,
                           w_z2, b_z2, w_r2, b_r2, w_h2, b_h2,
                           w_lin1, b_lin1, w_lin2, b_lin2)
    nc = _get_nc()
    res = run_bass_kernel_spmd(nc, in_maps, list(range(N_CORES))).results
    n = np.asarray(x).shape[0]
    full = unscramble([res[i]["out"] for i in range(N_CORES)])
    return np.ascontiguousarray(full[:n].reshape(n, 1).astype(np.float32))
